# revision 1
# baseline (speedup 1.0000x reference)
"""Trainium2 Bass kernel for DiffVAE assm scoring (segment softmax CE loss + acc).

Computation (see reference):
  x_pool = einsum("blh,kh->bk", x_mol_vecs, W_assm)        [32, 448]
  scores[t] = dot(x_pool[batch_idx[t]], cand_vecs[t])      [200000]
  per segment (25 cands): lse, label score, acc flag
  loss = sum(lse - label_score)/32 ; acc = mean(label >= segmax)

Sharding (per the sharding contract: candidates data-parallel, segments
whole, "replicate W_assm and the pooled x_mol_vecs"): 25000 cands = 1000
segments per core; the L-pooling of x_mol_vecs is host-side prep and the
pooled [448, 32] x_sum^T + W^T are replicated; each core applies W_assm
on-device. Per-core output is a [128, 16] tile of per-segment partial
losses and acc flags, summed on host.

Device strategy per core:
  - gather one-hot weights are generated ON CHIP: a 102KB batch-index
    table is loaded from HBM, partition-broadcast with one SBUF->SBUF DMA,
    and compared (is_equal) against the per-partition lane id - saves
    ~3.2MB of HBM traffic per core vs. loading the dense one-hot.
  - preamble (PE): x_pool = x_sum @ W^T via 4 K-chunk matmuls from the
    replicated x_sum^T; hi/lo-split and replicated to the 4x32 partition
    groups with SBUF->SBUF DMAs.
  - main loop over (seg_block 0..7, cand_slot 0..24):
      DMA cand tile [128 segs, 5 slots, 448] (contiguous 9KB/partition runs)
      PE: xg = onehot^T.T @ x_pool as TWO accumulating float32r matmuls
          over a hi/lo split of x_pool (hi = fp22-rounded, lo = remainder;
          the one-hot selects a single element so PSUM reconstructs the
          gathered x_pool row EXACTLY in fp32 - measured zero rel err vs
          the fp32 reference - at float32r speed, 1 cyc/row)
      ACT: copy xg PSUM->SBUF (keeps the DVE op off the PSUM-source
          init penalty; ACT is otherwise idle)
      DVE: fused multiply+reduce (scalar_tensor_tensor accum) -> scores
          column [128,1]
    scores accumulate into [128 segs, 25] tiles; segment softmax stats via
    DVE max-reduce + ACT exp-with-accum-sum + ACT ln; label select via
    one-hot fused multiply+reduce; acc flag via exact is_ge compare.

Sharding detail: 1000 segments/core processed as 8 uniform blocks of
125 rows (no partial block - the fused dots are free-dim-bound, so a
short last block would starve DMA while engines catch up).

Engine budget per core (cost-model): DMA ~133us (bound), ACT ~116us
(xg PSUM->SBUF moves + exp/ln), DVE ~111us (fused dots + one-hot gen),
PE ~88us; total ~142us model / ~135us expected on HW (the model prices
the SBUF->SBUF broadcast at HBM rate and serializes the ldweights/
matmul pairs that real HW overlaps). HBM traffic ~45.8MB/core (44.8MB
candidates + ~1MB tables; the one-hot index table and its broadcast run
as uint8 - DVE converts to fp32 for the exact is_equal compare).
"""

import numpy as np

import concourse.bass as bass
import concourse.tile as tile
from concourse import mybir
from concourse.bass_utils import run_bass_kernel_spmd

# problem constants (hardcoded per harness contract)
B, L, H = 32, 40, 448
S, NCAND = 8000, 25
T = S * NCAND
N_CORES = 8
TC = T // N_CORES          # 25000 candidates per core
SC = S // N_CORES          # 1000 segments per core
NBLK = 8                   # segment blocks
BROWS = SC // NBLK         # 125 rows per block (uniform - no partial block)
HCH = 112                  # h-chunk for preamble (448 = 4*112)
CCH = 5                    # cand slots per DMA chunk

f32 = mybir.dt.float32
f32r = mybir.dt.float32r
bf16 = mybir.dt.bfloat16
u8 = mybir.dt.uint8
Alu = mybir.AluOpType
Act = mybir.ActivationFunctionType


def _split_multi_waits(nc):
    """This walrus build only encodes a single sem-wait per instruction for
    several instruction classes (CTRL/Drain, S3_LW/ldweights, ...). Keep one
    wait on each instruction and move extras onto preceding NOPs issued on
    the same engine (engine queues are FIFO, so ordering is preserved)."""
    f = nc.m.functions[0]

    def make_nop(engine):
        nw = nc.engines[engine].nop().ins
        for b2 in f.blocks:
            if nw in b2.instructions:
                b2.instructions.remove(nw)
        return nw

    for bb in f.blocks:
        multi = [i for i in bb.instructions
                 if i.sync_info and len(i.sync_info.on_wait) > 1]
        for d in multi:
            waits = list(d.sync_info.on_wait)
            extra, keep = waits[:-1], waits[-1:]
            nops = []
            for w in extra:
                nw = make_nop(d.engine)
                nw.sync_info = mybir.SyncInfo(on_wait=[w], on_update=[])
                nops.append(nw)
            d.sync_info = mybir.SyncInfo(on_wait=keep,
                                         on_update=list(d.sync_info.on_update))
            idx = bb.instructions.index(d)
            bb.instructions[idx:idx] = nops


def build_bass():
    nc = bass.Bass("TRN2", target_bir_lowering=False, debug=False)

    cand = nc.dram_tensor("cand", [TC, H], f32, kind="ExternalInput").ap()
    xst = nc.dram_tensor("xst", [H, B], f32, kind="ExternalInput").ap()
    wt = nc.dram_tensor("wt", [H, H], f32, kind="ExternalInput").ap()
    bidxq = nc.dram_tensor("bidxq", [4, 6400], u8, kind="ExternalInput").ap()
    qmod = nc.dram_tensor("qmod", [128, 1], f32, kind="ExternalInput").ap()
    loh = nc.dram_tensor("loh", [128, NBLK, NCAND], u8, kind="ExternalInput").ap()
    out = nc.dram_tensor("out", [128, 2 * NBLK], f32, kind="ExternalOutput").ap()

    with tile.TileContext(nc) as tc:
        with (
            tc.tile_pool(name="singles", bufs=1) as singles,
            tc.tile_pool(name="pre_ps", bufs=1, space="PSUM") as pre_ps,
            tc.tile_pool(name="xg_ps", bufs=4, space="PSUM") as xg_ps,
            tc.tile_pool(name="cand_p", bufs=8) as cand_p,
            tc.tile_pool(name="xgs_p", bufs=4) as xgs_p,
            tc.tile_pool(name="ttro", bufs=2) as ttro_p,
            tc.tile_pool(name="sc_p", bufs=4) as sc_p,
            tc.tile_pool(name="small", bufs=10) as small,
            tc.tile_pool(name="ep", bufs=4) as ep,
        ):
            # ---- load replicated operands (preamble gates first, then
            #      first candidate block so DMA streams immediately) ----
            xsT_sb = singles.tile([HCH, 4, B], f32)
            nc.sync.dma_start(xsT_sb, xst.rearrange("(n p) b -> p n b", p=HCH))
            wt_sb = singles.tile([HCH, 4, H], f32)
            nc.sync.dma_start(wt_sb, wt.rearrange("(n p) k -> p n k", p=HCH))

            loh_sb = singles.tile([128, NBLK, NCAND], u8)
            cand_r = cand.rearrange("(s c) h -> s c h", c=NCAND)

            # one-hot gather weights generated on-chip: tiny batch-index
            # table from HBM, partition-broadcast via SBUF->SBUF DMA, then
            # DVE is_equal against the per-partition lane id. Generated in
            # two 64-row halves: rows 0:64 serve seg-blocks 0-3 (made during
            # the preamble), rows 64:128 serve blocks 4-7 (made during
            # block 0 when DVE has slack).
            bidx_sb = singles.tile([4, 6400], u8)
            nc.sync.dma_start(bidx_sb, bidxq)
            qmod_sb = singles.tile([128, 1], f32)
            nc.sync.dma_start(qmod_sb, qmod)
            bc_sb = singles.tile([128, 3200], u8)
            oht_sb = singles.tile([128, 6400], f32r)
            bap = bidx_sb[:]

            def gen_oht_colhalf(ch):
                # column-half ch covers seg-blocks {ch, ch+2, ch+4, ch+6};
                # the single [128, 3200] broadcast buffer is reused (WAR
                # serializes the two halves, which is fine - half 1 is only
                # needed from seg-block 1 onward)
                c0 = 3200 * ch
                part = bass.AP(tensor=bap.tensor, offset=bap.offset + c0,
                               ap=[[bap.ap[0][0], 4], [0, 32], [1, 3200]])
                nc.sync.dma_start(bc_sb, part)
                nc.vector.tensor_scalar(out=oht_sb[:, c0:c0 + 3200],
                                        in0=bc_sb,
                                        scalar1=qmod_sb[:],
                                        scalar2=None, op0=Alu.is_equal)

            gen_oht_colhalf(0)

            def issue_oht(k):
                nc.sync.dma_start(loh_sb[:, k, :], loh[:, k, :])

            border = list(range(NBLK))

            def issue_cand(k, last=False):
                rows = BROWS
                # finer trailing chunks on the final block shorten the
                # compute tail after the last DMA byte lands
                sizes = [5, 5, 5, 5, 3, 2] if last else [5, 5, 5, 5, 5]
                cts = []
                c0 = 0
                for n in sizes:
                    ct = cand_p.tile([128, CCH, H], f32, tag="ct", name="ct")
                    nc.sync.dma_start(
                        ct[:rows, :n, :],
                        cand_r[k * BROWS:k * BROWS + rows, c0:c0 + n, :],
                    )
                    cts.append((ct, c0, n))
                    c0 += n
                return cts

            issue_oht(border[0])
            pending = issue_cand(border[0])
            gen_oht_colhalf(1)

            out_sb = singles.tile([128, 2 * NBLK], f32)
            nc.vector.memset(out_sb, 0.0)

            # ---- preamble: x_pool = x_sum @ W^T (x_sum pooled on host per
            #      the sharding contract; replicated as [448, 32] = x_sum^T) ----
            pool_ps = pre_ps.tile([32, H], f32, tag="pool_ps")
            for jh in range(4):
                nc.tensor.matmul(
                    pool_ps,
                    lhsT=xsT_sb[:, jh, :],
                    rhs=wt_sb[:, jh, :],
                    start=(jh == 0), stop=(jh == 3),
                )
            # split x_pool = hi + lo with hi = fp22-rounded: the pair of
            # accumulating float32r one-hot matmuls then reconstructs the
            # gathered x_pool rows EXACTLY in fp32 (the one-hot selects a
            # single element, so PSUM adds hi[b*]+lo[b*] = x_pool[b*])
            xph_sb = singles.tile([128, H], f32r)
            nc.scalar.copy(xph_sb[0:32, :], pool_ps)
            xpf_sb = singles.tile([32, H], f32)
            nc.scalar.copy(xpf_sb, pool_ps)
            xlo_sb = singles.tile([32, H], f32)
            nc.vector.tensor_sub(xlo_sb, xpf_sb, xph_sb[0:32, :].bitcast(f32))
            xpl_sb = singles.tile([128, H], f32r)
            nc.scalar.copy(xpl_sb[0:32, :], xlo_sb)
            for q in range(1, 4):
                nc.sync.dma_start(xph_sb[32 * q:32 * q + 32, :],
                                  xph_sb[0:32, :])
                nc.sync.dma_start(xpl_sb[32 * q:32 * q + 32, :],
                                  xpl_sb[0:32, :])

            # ---- main loop ----
            for kord in range(NBLK):
                k = border[kord]
                rows = BROWS
                sc = sc_p.tile([128, NCAND], f32)
                cts = pending
                if kord + 1 < NBLK:
                    knext = border[kord + 1]
                    issue_oht(knext)
                    pending = issue_cand(knext, last=(kord + 1 == NBLK - 1))
                for ct, c0, n in cts:
                    for ci in range(n):
                        c = c0 + ci
                        g = k * NCAND + c
                        q, r = divmod(g, 50)
                        xg = xg_ps.tile([128, H], f32)
                        lhsT = oht_sb[32 * q:32 * q + 32,
                                      r * 128:r * 128 + rows]
                        nc.tensor.matmul(
                            xg[:rows], lhsT=lhsT,
                            rhs=xph_sb[32 * q:32 * q + 32, :],
                            start=True, stop=False,
                            tile_position=(32 * q, 0),
                        )
                        nc.tensor.matmul(
                            xg[:rows], lhsT=lhsT,
                            rhs=xpl_sb[32 * q:32 * q + 32, :],
                            start=False, stop=True,
                            tile_position=(32 * q, 0),
                        )
                        # ACT (otherwise idle) moves PSUM->SBUF so the DVE
                        # op avoids the PSUM-source init penalty
                        xgs = xgs_p.tile([128, H], f32)
                        nc.scalar.copy(xgs[:rows], xg[:rows])
                        ttro = ttro_p.tile([128, H], f32)
                        nc.vector.scalar_tensor_tensor(
                            out=ttro[:rows],
                            in0=ct[:rows, ci, :],
                            scalar=1.0,
                            in1=xgs[:rows],
                            op0=Alu.mult, op1=Alu.mult,
                            accum_out=sc[:rows, c:c + 1],
                        )
                # segment softmax stats for this block
                nm = small.tile([128, 1], f32)
                nc.vector.tensor_reduce(nm[:rows], sc[:rows, :],
                                        axis=mybir.AxisListType.X,
                                        op=Alu.max, negate=True)
                m = small.tile([128, 1], f32)
                nc.vector.tensor_scalar_mul(m[:rows], nm[:rows], -1.0)
                e = ep.tile([128, NCAND], f32)
                ssum = small.tile([128, 1], f32)
                nc.scalar.activation(e[:rows], sc[:rows, :], func=Act.Exp,
                                     bias=nm[:rows], scale=1.0,
                                     accum_out=ssum[:rows])
                ls = small.tile([128, 1], f32)
                nc.scalar.activation(ls[:rows], ssum[:rows], func=Act.Ln)
                lse = small.tile([128, 1], f32)
                nc.vector.tensor_sub(lse[:rows], ls[:rows], nm[:rows])
                lab = small.tile([128, 1], f32)
                ttro2 = ep.tile([128, NCAND], f32)
                nc.vector.scalar_tensor_tensor(
                    out=ttro2[:rows],
                    in0=sc[:rows, :],
                    scalar=1.0,
                    in1=loh_sb[:rows, k, :],
                    op0=Alu.mult, op1=Alu.mult,
                    accum_out=lab[:rows],
                )
                nc.vector.tensor_sub(out_sb[:rows, k:k + 1], lse[:rows], lab[:rows])
                nc.vector.tensor_tensor(out_sb[:rows, NBLK + k:NBLK + k + 1],
                                        lab[:rows], m[:rows], op=Alu.is_ge)

            nc.sync.dma_start(out, out_sb)

    _split_multi_waits(nc)
    return nc


def make_inputs(x_mol_vecs, cand_vecs, W_assm, batch_idx, label_in_seg):
    """Host-side shard + index preprocessing. Returns per-core input maps."""
    # pooling over L is host-side prep per the sharding contract
    # ("replicate ... the pooled x_mol_vecs"); replicated as x_sum^T
    xs = np.asarray(x_mol_vecs, np.float32).sum(axis=1, dtype=np.float32)
    xst = np.ascontiguousarray(xs.T)
    cand = np.asarray(cand_vecs, np.float32)
    W = np.asarray(W_assm, np.float32)
    bi = np.asarray(batch_idx).astype(np.int64)
    lab = np.asarray(label_in_seg).astype(np.int64)

    wt = np.ascontiguousarray(W.T)
    qmod = (np.arange(128) % 32).astype(np.float32).reshape(128, 1)

    in_maps = []
    for core in range(N_CORES):
        s0 = core * SC
        bi_c = bi[core * TC:(core + 1) * TC].reshape(SC, NCAND)
        lab_c = lab[s0:s0 + SC]

        bidxq = np.zeros((4, 6400), np.uint8)
        for g in range(NBLK * NCAND):
            k, cc = divmod(g, NCAND)
            q, rr = divmod(g, 50)
            segs = np.arange(BROWS) + k * BROWS
            bidxq[q, rr * 128 + np.arange(BROWS)] = bi_c[segs, cc]

        loh = np.zeros((128, NBLK, NCAND), np.uint8)
        segs = np.arange(SC)
        loh[segs % BROWS, segs // BROWS, lab_c] = 1.0

        in_maps.append({
            "cand": np.ascontiguousarray(cand[core * TC:(core + 1) * TC]),
            "xst": xst,
            "wt": wt,
            "bidxq": bidxq,
            "qmod": qmod,
            "loh": loh,
        })
    return in_maps


_NC_CACHE = None


def kernel(x_mol_vecs, cand_vecs, W_assm, batch_idx, label_in_seg,
           ncand=NCAND, num_segments=S, **_ignored):
    global _NC_CACHE
    assert int(ncand) == NCAND and int(num_segments) == S

    in_maps = make_inputs(x_mol_vecs, cand_vecs, W_assm, batch_idx, label_in_seg)
    if _NC_CACHE is None:
        _NC_CACHE = build_bass()
    res = run_bass_kernel_spmd(_NC_CACHE, in_maps, core_ids=list(range(N_CORES)))

    loss_sum = 0.0
    acc_sum = 0.0
    for core in range(N_CORES):
        o = res.results[core]["out"]
        loss_sum += float(o[:, :NBLK].sum(dtype=np.float64))
        acc_sum += float(o[:, NBLK:].sum(dtype=np.float64))
    loss = np.float32(loss_sum / B)
    acc = np.float32(acc_sum / S)
    return loss, acc



# revision 2
# speedup vs baseline: 3.1124x; 3.1124x over previous
"""Trainium2 Bass kernel for DiffVAE assm scoring (segment softmax CE loss + acc).

Computation (see reference):
  x_pool = einsum("blh,kh->bk", x_mol_vecs, W_assm)        [32, 448]
  scores[t] = dot(x_pool[batch_idx[t]], cand_vecs[t])      [200000]
  per segment (25 cands): lse, label score, acc flag
  loss = sum(lse - label_score)/32 ; acc = mean(label >= segmax)

Sharding (per the sharding contract: candidates data-parallel, segments
whole, "replicate W_assm and the pooled x_mol_vecs"): 25000 cands = 1000
segments per core; the L-pooling of x_mol_vecs is host-side prep and the
pooled x_sum^T + W^T are replicated (in bf16); each core applies W_assm
on-device. Per-core output is a [128, 16] tile of per-segment partial
losses and acc flags, summed on host.

The kernel is memory-bound on the candidate stream, so candidates are
uploaded as fp8 e4m3 (11.2MB/core vs 44.8MB fp32) and fed to the PE as
stationary weights; x_pool^T (bf16) is the moving operand. Measured on
the exact seed-0 data, the fp8 candidate + bf16 x-path quantization gives
loss rel err 4.2e-4 and acc rel err 6.3e-3 (24 raw sign flips of the
label>=max compare, net -2 of 316), comfortably under the 2e-2 gate.

Device strategy per core (1000 segments as 8 blocks of 125 rows):
  - preamble (PE): x_pool^T = W @ x_sum^T via 16 bf16 matmuls
    ([112,112] W^T chunks x [112,32] x_sum^T chunks -> 4 PSUM col groups);
    ACT copies PSUM -> bf16 SBUF [112, 4, 32].
  - main loop over seg blocks k:
      DMA candT tile [112, 4hq, 3125] fp8 (contiguous 3125B runs; host
        pre-transposed to [h, block, slot, seg] order)
      PE: per slot (25): 4 accumulating matmuls
        psum[125 seg, 32 b] += candT[:, q, slot]^T @ xpT[:, q, :]
        (fp8 stationary x bf16 moving -> 1 cyc/row, 32 rows each)
      DVE: one-hot select of score[seg, slot] = psum[seg, batch_idx]:
        multiply psum by host-built u8 one-hot [125, slot, 32] then
        tensor_reduce over the 32-wide X axis -> sc[125, 25]
      segment softmax stats: DVE max-reduce, ACT exp-with-accum-sum + ln,
      label select via one-hot fused multiply-reduce, acc via is_ge.

Engine budget per core (cost-model): DMA ~35us (bound: 31.1us fp8
candidates + oh/W/misc), DVE ~20us, PE ~11-21us (p-state dependent),
ACT ~4us. HBM traffic ~12.5MB/core.
"""

import numpy as np
import ml_dtypes

import concourse.bass as bass
import concourse.tile as tile
from concourse import mybir
from concourse.bass_utils import run_bass_kernel_spmd

# problem constants (hardcoded per harness contract)
B, L, H = 32, 40, 448
S, NCAND = 8000, 25
T = S * NCAND
N_CORES = 8
TC = T // N_CORES          # 25000 candidates per core
SC = S // N_CORES          # 1000 segments per core
NBLK = 8                   # segment blocks
BROWS = SC // NBLK         # 125 rows per block
HCH = 112                  # h-chunk (448 = 4*112)
NQ = 4                     # h-chunks
BLKX = NCAND * BROWS       # 3125 candidates per block

f32 = mybir.dt.float32
bf16 = mybir.dt.bfloat16
f8e4 = mybir.dt.float8e4
u8 = mybir.dt.uint8
Alu = mybir.AluOpType
Act = mybir.ActivationFunctionType

NP_F8 = ml_dtypes.float8_e4m3
NP_BF16 = ml_dtypes.bfloat16


def _split_multi_waits(nc):
    """This walrus build only encodes a single sem-wait per instruction for
    several instruction classes (CTRL/Drain, S3_LW/ldweights, ...). Keep one
    wait on each instruction and move extras onto preceding NOPs issued on
    the same engine (engine queues are FIFO, so ordering is preserved)."""
    f = nc.m.functions[0]

    def make_nop(engine):
        nw = nc.engines[engine].nop().ins
        for b2 in f.blocks:
            if nw in b2.instructions:
                b2.instructions.remove(nw)
        return nw

    for bb in f.blocks:
        multi = [i for i in bb.instructions
                 if i.sync_info and len(i.sync_info.on_wait) > 1]
        for d in multi:
            waits = list(d.sync_info.on_wait)
            extra, keep = waits[:-1], waits[-1:]
            nops = []
            for w in extra:
                nw = make_nop(d.engine)
                nw.sync_info = mybir.SyncInfo(on_wait=[w], on_update=[])
                nops.append(nw)
            d.sync_info = mybir.SyncInfo(on_wait=keep,
                                         on_update=list(d.sync_info.on_update))
            idx = bb.instructions.index(d)
            bb.instructions[idx:idx] = nops


def build_bass():
    nc = bass.Bass("TRN2", target_bir_lowering=False, debug=False)

    candq = nc.dram_tensor("candq", [NBLK, NQ, HCH, BLKX], f8e4,
                           kind="ExternalInput").ap()
    xst = nc.dram_tensor("xst", [H, B], bf16, kind="ExternalInput").ap()
    wt = nc.dram_tensor("wt", [H, H], bf16, kind="ExternalInput").ap()
    oh = nc.dram_tensor("oh", [128, NBLK, NCAND, B], u8,
                        kind="ExternalInput").ap()
    loh = nc.dram_tensor("loh", [128, NBLK, NCAND], u8,
                         kind="ExternalInput").ap()
    out = nc.dram_tensor("out", [128, 2 * NBLK], f32, kind="ExternalOutput").ap()

    cand_r = candq.rearrange("k q p x -> p k q x")   # [112, 8, 4, 3125]

    with tile.TileContext(nc) as tc:
        with (
            tc.tile_pool(name="singles", bufs=1) as singles,
            tc.tile_pool(name="pre_ps", bufs=1, space="PSUM") as pre_ps,
            tc.tile_pool(name="psA", bufs=2, space="PSUM") as psA_p,
            tc.tile_pool(name="psB", bufs=2, space="PSUM") as psB_p,
            tc.tile_pool(name="cand_p", bufs=2) as cand_p,
            tc.tile_pool(name="oh_p", bufs=2) as oh_p,
            tc.tile_pool(name="tmp_p", bufs=4) as tmp_p,
            tc.tile_pool(name="sc_p", bufs=2) as sc_p,
            tc.tile_pool(name="small", bufs=10) as small,
            tc.tile_pool(name="ep", bufs=4) as ep,
        ):
            # ---- replicated operands; preamble gates first, then the first
            #      candidate block so DMA streams immediately ----
            xsT_sb = singles.tile([HCH, NQ, B], bf16)
            nc.sync.dma_start(xsT_sb, xst.rearrange("(n p) b -> p n b", p=HCH))
            wt_sb = singles.tile([HCH, NQ, H], bf16)
            nc.sync.dma_start(wt_sb, wt.rearrange("(n p) k -> p n k", p=HCH))
            loh_sb = singles.tile([128, NBLK, NCAND], u8)
            nc.sync.dma_start(loh_sb, loh)

            def issue_oh(k):
                t = oh_p.tile([128, NCAND, B], u8, tag="oh", name="oh")
                nc.sync.dma_start(t, oh[:, k, :, :])
                return t

            def issue_cand(k, last=False):
                # sub-chunk in candidate-slot groups: finer trailing chunks on
                # the final block shorten the compute tail after the last DMA
                # byte lands
                sizes = [5, 5, 5, 5, 5] if last else [12, 13]
                ct = cand_p.tile([HCH, NQ, BLKX], f8e4, tag="ct", name="ct")
                c0 = 0
                for n in sizes:
                    j0, j1 = c0 * BROWS, (c0 + n) * BROWS
                    nc.sync.dma_start(ct[:, :, j0:j1], cand_r[:, k, :, j0:j1])
                    c0 += n
                return ct

            oh_pend = issue_oh(0)
            ct_pend = issue_cand(0)

            out_sb = singles.tile([128, 2 * NBLK], f32)
            nc.vector.memset(out_sb, 0.0)

            # ---- preamble: x_pool^T = W @ x_sum^T, all bf16 ----
            xpT_ps = pre_ps.tile([HCH, 4 * B], f32, tag="xpT_ps")
            for kq in range(4):
                for hq in range(4):
                    nc.tensor.matmul(
                        xpT_ps[:, kq * B:(kq + 1) * B],
                        lhsT=wt_sb[:, hq, kq * HCH:(kq + 1) * HCH],
                        rhs=xsT_sb[:, hq, :],
                        start=(hq == 0), stop=(hq == 3),
                    )
            xpT_sb = singles.tile([HCH, NQ, B], bf16)
            nc.scalar.copy(xpT_sb, xpT_ps)

            # ---- main loop over segment blocks ----
            for k in range(NBLK):
                rows = BROWS
                ct, oht = ct_pend, oh_pend
                if k + 1 < NBLK:
                    oh_pend = issue_oh(k + 1)
                    ct_pend = issue_cand(k + 1, last=(k + 1 == NBLK - 1))

                # PE: all-batch scores, 16 slots in psA, 9 in psB
                psA = psA_p.tile([128, 16, B], f32, tag="psA")
                psB = psB_p.tile([128, 9, B], f32, tag="psB")
                for slot in range(NCAND):
                    ps, srow = (psA, slot) if slot < 16 else (psB, slot - 16)
                    for q in range(NQ):
                        nc.tensor.matmul(
                            ps[:rows, srow, :],
                            lhsT=ct[:, q, slot * BROWS:(slot + 1) * BROWS],
                            rhs=xpT_sb[:, q, :],
                            start=(q == 0), stop=(q == 3),
                        )

                # DVE: one-hot select -> sc [125, 25]
                sc = sc_p.tile([128, NCAND], f32, tag="sc")
                tmpA = tmp_p.tile([128, 16, B], f32, tag="tmpA")
                nc.vector.scalar_tensor_tensor(
                    out=tmpA[:rows], in0=psA[:rows], scalar=1.0,
                    in1=oht[:rows, 0:16, :], op0=Alu.mult, op1=Alu.mult)
                nc.vector.tensor_reduce(sc[:rows, 0:16], tmpA[:rows],
                                        axis=mybir.AxisListType.X, op=Alu.add)
                tmpB = tmp_p.tile([128, 9, B], f32, tag="tmpB")
                nc.vector.scalar_tensor_tensor(
                    out=tmpB[:rows], in0=psB[:rows], scalar=1.0,
                    in1=oht[:rows, 16:25, :], op0=Alu.mult, op1=Alu.mult)
                nc.vector.tensor_reduce(sc[:rows, 16:25], tmpB[:rows],
                                        axis=mybir.AxisListType.X, op=Alu.add)

                # segment softmax stats for this block
                nm = small.tile([128, 1], f32)
                nc.vector.tensor_reduce(nm[:rows], sc[:rows, :],
                                        axis=mybir.AxisListType.X,
                                        op=Alu.max, negate=True)
                m = small.tile([128, 1], f32)
                nc.vector.tensor_scalar_mul(m[:rows], nm[:rows], -1.0)
                e = ep.tile([128, NCAND], f32)
                ssum = small.tile([128, 1], f32)
                nc.scalar.activation(e[:rows], sc[:rows, :], func=Act.Exp,
                                     bias=nm[:rows], scale=1.0,
                                     accum_out=ssum[:rows])
                ls = small.tile([128, 1], f32)
                nc.scalar.activation(ls[:rows], ssum[:rows], func=Act.Ln)
                lse = small.tile([128, 1], f32)
                nc.vector.tensor_sub(lse[:rows], ls[:rows], nm[:rows])
                lab = small.tile([128, 1], f32)
                ttro2 = ep.tile([128, NCAND], f32)
                nc.vector.scalar_tensor_tensor(
                    out=ttro2[:rows],
                    in0=sc[:rows, :],
                    scalar=1.0,
                    in1=loh_sb[:rows, k, :],
                    op0=Alu.mult, op1=Alu.mult,
                    accum_out=lab[:rows],
                )
                nc.vector.tensor_sub(out_sb[:rows, k:k + 1], lse[:rows], lab[:rows])
                nc.vector.tensor_tensor(out_sb[:rows, NBLK + k:NBLK + k + 1],
                                        lab[:rows], m[:rows], op=Alu.is_ge)

            nc.sync.dma_start(out, out_sb)

    _split_multi_waits(nc)
    return nc


def make_inputs(x_mol_vecs, cand_vecs, W_assm, batch_idx, label_in_seg):
    """Host-side shard + quantize + index preprocessing. Per-core input maps."""
    # pooling over L is host-side prep per the sharding contract
    # ("replicate ... the pooled x_mol_vecs"); replicated as x_sum^T in bf16
    xs = np.asarray(x_mol_vecs, np.float32).sum(axis=1, dtype=np.float32)
    xst = np.ascontiguousarray(xs.T.astype(NP_BF16))
    cand = np.asarray(cand_vecs, np.float32)
    W = np.asarray(W_assm, np.float32)
    bi = np.asarray(batch_idx).astype(np.int64)
    lab = np.asarray(label_in_seg).astype(np.int64)

    wt = np.ascontiguousarray(W.T.astype(NP_BF16))

    in_maps = []
    for core in range(N_CORES):
        s0 = core * SC
        cc = cand[core * TC:(core + 1) * TC].astype(NP_F8)   # [25000, 448]
        # device layout [block, hq, hp, slot*125+row]: candidate
        # t_local = (block*125+row)*25 + slot, h = hq*112+hp
        v = cc.reshape(NBLK, BROWS, NCAND, H)                # [k, r, slot, h]
        v = v.transpose(3, 0, 2, 1)                          # [h, k, slot, r]
        candq = np.ascontiguousarray(
            v.reshape(NQ, HCH, NBLK, BLKX).transpose(2, 0, 1, 3))

        bi_c = bi[core * TC:(core + 1) * TC].reshape(NBLK, BROWS, NCAND)
        ohc = np.zeros((128, NBLK, NCAND, B), np.uint8)
        kk, rr, ss = np.meshgrid(np.arange(NBLK), np.arange(BROWS),
                                 np.arange(NCAND), indexing="ij")
        ohc[rr, kk, ss, bi_c] = 1

        lab_c = lab[s0:s0 + SC]
        lohc = np.zeros((128, NBLK, NCAND), np.uint8)
        segs = np.arange(SC)
        lohc[segs % BROWS, segs // BROWS, lab_c] = 1

        in_maps.append({
            "candq": candq,
            "xst": xst,
            "wt": wt,
            "oh": ohc,
            "loh": lohc,
        })
    return in_maps


_NC_CACHE = None


def kernel(x_mol_vecs, cand_vecs, W_assm, batch_idx, label_in_seg,
           ncand=NCAND, num_segments=S, **_ignored):
    global _NC_CACHE
    assert int(ncand) == NCAND and int(num_segments) == S

    in_maps = make_inputs(x_mol_vecs, cand_vecs, W_assm, batch_idx, label_in_seg)
    if _NC_CACHE is None:
        _NC_CACHE = build_bass()
    res = run_bass_kernel_spmd(_NC_CACHE, in_maps, core_ids=list(range(N_CORES)))

    loss_sum = 0.0
    acc_sum = 0.0
    for core in range(N_CORES):
        o = res.results[core]["out"]
        loss_sum += float(o[:, :NBLK].sum(dtype=np.float64))
        acc_sum += float(o[:, NBLK:].sum(dtype=np.float64))
    loss = np.float32(loss_sum / B)
    acc = np.float32(acc_sum / S)
    return loss, acc
